# revision 6
# baseline (speedup 1.0000x reference)
# GCN layer kernel for Trainium2: out[b] = relu((a[b] @ x[b]) @ W) * mask[b]
#
# Sharding: data-parallel over the batch (graph) dim. B=8 graphs, 8 cores,
# one graph per core; W replicated. Inputs are the FULL tensors; shards are
# sliced host-side and the per-core outputs stacked back together.
#
# Per-core dataflow (a: [2048,2048], x: [2048,512], W: [512,512]):
#   - All matmuls in bf16 (fp32 PSUM accumulation); rel-err ~3e-3, tol 2e-2.
#   - a must be used transposed (contraction over its column index m).
#     Chunk 0's 64 [128,128] tiles transpose on the PE (idle during the
#     initial DMA window anyway); chunks 1-3 transpose via the DMA xbar
#     (InstDmaTransposeAnt, 16x128 tiles) on the scalar HWDGE ring,
#     writing straight into at[p, mi, ni, j] = a[n0+j, 128*mi+p]. This
#     keeps the PE stream pure N=512 matmuls (no HAM-invisible transpose
#     phases, no PSUM->SBUF copybacks after chunk 0).
#   - mm1 (mi-outer): pt[fi] = tT[fi-block, n-chunk] accumulated over mi
#     with lhsT = x[mi, fi-block], rhs = at[:, mi]. mi-outer lets the x
#     row-tiles trickle in behind chunk 0's strips instead of gating the
#     start of mm1. mm2: out[n-tile, d] = sum_fi tT[fi, n-tile]^T @ W[fi],
#     two 2-bank waves per chunk riding the next chunk's mm1 stream.
#   - Engine assignment: sync ring = all loads (x0,x1, c0, x2..15, c1,
#     c2, c3) then chunk-2/3 stores; scalar ring = xbar transposes +
#     chunk-0/1 stores; scalar compute = |x| mask + ReLU; DVE = casts +
#     chunk-0 transpose copybacks + mask compare; gpsimd = W cast-DMA +
#     tT PSUM->SBUF copies (scalar/DVE streams are blocked behind xbar/
#     cast waits right when tT is ready - gpsimd is free).
#   - HAM: identity warms up front plus warm matmuls chained to chunk-0
#     strip casts keep the PE activity window busy through the 0..~15us
#     DMA window (PE transposes do NOT count as HAM activity).

import numpy as np

B, N, F, D = 8, 2048, 512, 512
P = 128
NT = N // P        # 16 row-tiles of n (and of m, since a is square)
FT = F // P        # 4 tiles of f
NCHUNK = 512       # n is processed in chunks of 512 rows
NJ = N // NCHUNK   # 4
NSUB = NCHUNK // P # 4

_CACHE = {}


def _build_nc():
    from contextlib import ExitStack

    from concourse import bacc, mybir, tile
    from concourse.masks import make_identity

    f32 = mybir.dt.float32
    bf16 = mybir.dt.bfloat16
    AF = mybir.ActivationFunctionType

    nc = bacc.Bacc(None)
    a_d = nc.dram_tensor("a", [N, N], f32, kind="ExternalInput")
    x_d = nc.dram_tensor("x", [N, F], f32, kind="ExternalInput")
    w_d = nc.dram_tensor("kernel", [F, D], f32, kind="ExternalInput")
    o_d = nc.dram_tensor("out", [N, D], f32, kind="ExternalOutput")

    with tile.TileContext(nc) as tc, ExitStack() as ctx:
        const = ctx.enter_context(tc.tile_pool(name="const", bufs=1))
        xp = ctx.enter_context(tc.tile_pool(name="xp", bufs=1))
        wp = ctx.enter_context(tc.tile_pool(name="wp", bufs=1))
        xlp = ctx.enter_context(tc.tile_pool(name="xlp", bufs=4))
        afp = ctx.enter_context(tc.tile_pool(name="afp", bufs=3))
        abp = ctx.enter_context(tc.tile_pool(name="abp", bufs=6))
        atp = ctx.enter_context(tc.tile_pool(name="atp", bufs=3))
        ttp = ctx.enter_context(tc.tile_pool(name="ttp", bufs=2))
        outp = ctx.enter_context(tc.tile_pool(name="outp", bufs=3))
        scr = ctx.enter_context(tc.tile_pool(name="scr", bufs=2))
        ps_mm = ctx.enter_context(tc.tile_pool(name="ps_mm", bufs=4, space="PSUM"))
        ps_o = ctx.enter_context(tc.tile_pool(name="ps_o", bufs=2, space="PSUM"))
        ps_tp = ctx.enter_context(tc.tile_pool(name="ps_tp", bufs=2, space="PSUM"))

        ident = const.tile([P, P], f32)
        make_identity(nc, ident[:])
        ident_b = const.tile([P, P], bf16)
        nc.vector.tensor_copy(ident_b[:], ident[:])

        x_b = xp.tile([P, NT, F], bf16)
        w_b = wp.tile([P, FT, D], bf16)
        sumabs = const.tile([P, NT], f32)
        mask_sb = const.tile([P, NT], f32)

        def warm_fp32():
            pw = ps_o.tile([P, D], f32, tag="pso", name="pw")
            nc.tensor.matmul(
                pw[:, :P], lhsT=ident[:], rhs=ident[:], start=True, stop=True
            )

        def warm_bf16(lhs, rhs):
            # fires as the just-cast tile lands; paces PE activity (HAM)
            # through the DMA window.
            pw = ps_o.tile([P, D], f32, tag="pso", name="pwb")
            nc.tensor.matmul(
                pw[:, : rhs.shape[-1]], lhsT=lhs, rhs=rhs, start=True, stop=True
            )

        at_tiles = [None] * NJ

        def at_of(nj):
            if at_tiles[nj] is None:
                at_tiles[nj] = atp.tile(
                    [P, NT, NSUB, P], bf16, tag="at", name=f"at{nj}"
                )
            return at_tiles[nj]

        # ---------- preamble: loads (sync ring order = priority) ----------
        xls = [None] * NT

        def load_x(o):
            xl = xlp.tile([P, F], f32, tag="xl", name=f"xl{o}")
            nc.sync.dma_start(xl[:], x_d[o * P : (o + 1) * P, :])
            xls[o] = xl

        def load_strip(nj, ni):
            af = afp.tile([P, N], f32, tag="af", name="af")
            r0 = (nj * NSUB + ni) * P
            nc.sync.dma_start(af[:], a_d[r0 : r0 + P, :])
            return af

        for _ in range(10):
            warm_fp32()

        load_x(0)
        load_x(1)
        af0 = [load_strip(0, ni) for ni in range(NSUB)]
        for o in range(2, NT):
            load_x(o)
        af1 = [load_strip(1, ni) for ni in range(NSUB)]
        af2 = [load_strip(2, ni) for ni in range(NSUB)]
        af3 = [load_strip(3, ni) for ni in range(NSUB)]
        nc.gpsimd.dma_start(w_b[:], w_d[:].rearrange("(o p) d -> p o d", p=P))

        # ---------- preamble: chunk 0 cast + PE transpose ----------
        def cast_x(o):
            nc.vector.tensor_copy(x_b[:, o, :], xls[o][:])

        cast_x(0)
        cast_x(1)
        warm_bf16(x_b[:, 0, 0:P], x_b[:, 0, :])
        warm_bf16(x_b[:, 1, 0:P], x_b[:, 1, :])

        def cast_strip(nj, ni, af, warm=False):
            ab = abp.tile([P, N], bf16, tag="ab", name=f"ab{nj}_{ni}")
            nc.vector.tensor_copy(ab[:], af[:])
            if warm:
                warm_bf16(ab[:, 0:P], ab[:, 0:NCHUNK])
                warm_bf16(ab[:, P : 2 * P], ab[:, NCHUNK : 2 * NCHUNK])
            return ab

        def t_quads_pe(nj, ni, ab):
            # PE-transpose strip ni (16 [128,128] tiles) through PSUM in
            # quads; DVE copies into at[p, mi, ni, :].
            at = at_of(nj)
            for q in range(4):
                ps = ps_tp.tile([P, NCHUNK], bf16, tag="pst", name="pst")
                for k in range(4):
                    mi = q * 4 + k
                    nc.tensor.transpose(
                        ps[:, k * P : (k + 1) * P],
                        ab[:, mi * P : (mi + 1) * P],
                        ident_b[:],
                    )
                nc.vector.tensor_copy(
                    at[:, q * 4 : (q + 1) * 4, ni, :],
                    ps[:].rearrange("p (a f) -> p a f", a=4),
                )

        for ni in range(NSUB):
            ab = cast_strip(0, ni, af0[ni], warm=True)
            t_quads_pe(0, ni, ab)

        for o in range(2, NT):
            cast_x(o)

        # mask: |x| row-sums via ACT (first in the ACT stream, ahead of the
        # xbar waits), mask = (sum > 0) on DVE.
        for o in range(NT):
            abs_scr = scr.tile([P, F], bf16, tag="abs_scr")
            nc.scalar.activation(
                abs_scr[:], x_b[:, o, :], AF.Abs, accum_out=sumabs[:, o : o + 1]
            )
        nc.vector.tensor_scalar(
            mask_sb[:], sumabs[:], 0.0, None, mybir.AluOpType.is_gt
        )

        # chunk 1+ cast (DVE) + xbar transpose (scalar ring)
        def xbar_strip(nj, ni, ab):
            nc.scalar.dma_start(at_of(nj)[:, :, ni, :], ab[:], transpose=True)

        def cast_xbar_chunk(nj, afs, nis=range(NSUB)):
            for ni in nis:
                ab = cast_strip(nj, ni, afs[ni])
                xbar_strip(nj, ni, ab)

        cast_xbar_chunk(1, af1, nis=(0, 1))

        # ---------- main loop ----------
        tts = [None] * NJ

        def mm1_chunk(nj, mm2_at=None):
            # mi-outer: 4 fi banks accumulate across all 16 mi steps.
            at = at_of(nj)
            pt = [
                ps_mm.tile([P, NCHUNK], f32, tag="psm", name=f"pt_{nj}_{fi}")
                for fi in range(FT)
            ]
            for mi in range(NT):
                for fi in range(FT):
                    nc.tensor.matmul(
                        pt[fi][:],
                        lhsT=x_b[:, mi, fi * P : (fi + 1) * P],
                        rhs=at[:, mi, :, :],
                        start=(mi == 0),
                        stop=(mi == NT - 1),
                    )
                if mm2_at is not None and mi in mm2_at:
                    mm2_wave(nj - 1, mm2_at[mi])
            # tT PSUM->SBUF on DVE, positioned in the DVE stream between
            # the next chunk's strip casts so it isn't blocked behind them.
            tt = ttp.tile([P, FT, NCHUNK], bf16, tag="tt", name=f"tt{nj}")
            tts[nj] = tt
            for fi in range(FT):
                nc.vector.tensor_copy(tt[:, fi], pt[fi][:])

        def mm2_wave(nj, w):
            # half of mm2 for chunk nj: output tiles ns = 2w, 2w+1,
            # accumulated over fi in 2 PSUM banks (alternating pools to
            # avoid WAR stalls on the previous wave's ReLU), then fused
            # relu(mask * po) -> SBUF -> store (2 row-tiles per DMA).
            tt = tts[nj]
            pool, tg = (ps_o, "pso") if w == 0 else (ps_tp, "pst")
            pos = [
                pool.tile([P, D], f32, tag=tg, name=f"po_{nj}_{w}_{i}")
                for i in range(2)
            ]
            for fi in range(FT):
                for i in range(2):
                    ns = 2 * w + i
                    nc.tensor.matmul(
                        pos[i][:],
                        lhsT=tt[:, fi, ns * P : (ns + 1) * P],
                        rhs=w_b[:, fi],
                        start=(fi == 0),
                        stop=(fi == FT - 1),
                    )
            ob = outp.tile([P, 2, D], f32, tag="ob")
            for i in range(2):
                ni = nj * NSUB + 2 * w + i
                nc.scalar.activation(
                    ob[:, i, :], pos[i][:], AF.Relu, scale=mask_sb[:, ni : ni + 1]
                )
            r0 = (nj * NSUB + 2 * w) * P
            dst = o_d[r0 : r0 + 2 * P, :].rearrange("(t p) d -> p t d", p=P)
            q = nc.scalar if nj < 2 else nc.sync
            q.dma_start(dst, ob[:])

        # chunk 0: pure mm1 (chunk 1 s0/s1 xbars already in flight)
        mm1_chunk(0)
        cast_xbar_chunk(1, af1, nis=(2, 3))
        # fill the wait for chunk 1's at with chunk 0's mm2
        mm2_wave(0, 0)
        mm2_wave(0, 1)

        cast_xbar_chunk(2, af2)
        mm1_chunk(1)
        cast_xbar_chunk(3, af3)
        mm1_chunk(2, mm2_at={4: 0, 8: 1})
        mm1_chunk(3, mm2_at={4: 0, 8: 1})
        mm2_wave(3, 0)
        mm2_wave(3, 1)

    nc.compile()
    return nc


def get_nc():
    if "nc" not in _CACHE:
        _CACHE["nc"] = _build_nc()
    return _CACHE["nc"]


def kernel(**inputs) -> np.ndarray:
    from concourse.bass_utils import run_bass_kernel_spmd

    x = np.ascontiguousarray(np.asarray(inputs["x"], dtype=np.float32))
    a = np.ascontiguousarray(np.asarray(inputs["a"], dtype=np.float32))
    w = np.ascontiguousarray(np.asarray(inputs["kernel"], dtype=np.float32))
    assert x.shape == (B, N, F) and a.shape == (B, N, N) and w.shape == (F, D)

    nc = get_nc()
    in_maps = [{"a": a[b], "x": x[b], "kernel": w} for b in range(B)]
    res = run_bass_kernel_spmd(nc, in_maps, core_ids=list(range(B)))
    return np.stack([res.results[b]["out"] for b in range(B)], axis=0)


# revision 8
# speedup vs baseline: 1.2961x; 1.2961x over previous
# GCN layer kernel for Trainium2: out[b] = relu((a[b] @ x[b]) @ W) * mask[b]
#
# Sharding: data-parallel over the batch (graph) dim. B=8 graphs, 8 cores,
# one graph per core; W replicated. Inputs are the FULL tensors; shards are
# sliced host-side and the per-core outputs stacked back together.
#
# Per-core dataflow (a: [2048,2048], x: [2048,512], W: [512,512]):
#   - All matmuls in bf16 (fp32 PSUM accumulation); rel-err ~3e-3, tol 2e-2.
#   - a is used transposed (contraction over its column index m): strips
#     cast to bf16 on DVE, PE-transposed in quads of [128,128] tiles
#     through PSUM, copied back into at[p, mi, ni, j] = a[n0+j, 128mi+p]
#     with the copyback alternating DVE/ACT. (A DMA-xbar variant was
#     measured: InstDmaTransposeAnt shatters into 320B packets that choke
#     the SDMA fabric for ~10us/strip - PE transposes in a dense matmul
#     stream cost only ~50ns each.)
#   - Load order (sync ring, ~358 GB/s HBM cap): x0, x1, c0 strips,
#     x2..x15, c1, c2, c3 strips. mm1 for chunk 0 is mi-outer so the x
#     row-tiles can trickle in BEHIND chunk 0's strips instead of gating
#     the start (the classic fi-outer form needs all of x up front):
#     mm1 c0 starts ~15us, right as chunk 0 finishes transposing.
#   - Chunks 1-3 run fi-outer (baseline-proven pipeline): pt[fi] = tT
#     accumulated over mi; the NEXT chunk's strip ni is cast after the
#     fi==ni group and its 4 transpose-quads ride the fi==ni+1 matmul
#     stream; mm2 waves of the previous chunk run after fi==1 / fi==3.
#     Chunk 3's mm2 is a single fi-major merged wave across 4 PSUM banks
#     (ps_o + ps_tp, quads done by then) to minimize the tail.
#   - w rides gpsimd as a cast-DMA; stores go 2 row-tiles per DMA,
#     chunks 0-1 on the scalar ring mid-run, chunks 2-3 on the sync ring
#     after the loads have drained.
#   - HAM: identity warms up front plus warm matmuls chained to chunk-0
#     strip/x casts pace PE activity through the 0..15us DMA window (PE
#     transposes do NOT count as HAM activity; straggler warms cover the
#     transpose-heavy preamble).

import numpy as np

B, N, F, D = 8, 2048, 512, 512
P = 128
NT = N // P        # 16 row-tiles of n (and of m, since a is square)
FT = F // P        # 4 tiles of f
NCHUNK = 512       # n is processed in chunks of 512 rows
NJ = N // NCHUNK   # 4
NSUB = NCHUNK // P # 4

_CACHE = {}


def _build_nc():
    from contextlib import ExitStack

    from concourse import bacc, mybir, tile
    from concourse.masks import make_identity

    f32 = mybir.dt.float32
    bf16 = mybir.dt.bfloat16
    AF = mybir.ActivationFunctionType

    nc = bacc.Bacc(None)
    a_d = nc.dram_tensor("a", [N, N], f32, kind="ExternalInput")
    x_d = nc.dram_tensor("x", [N, F], f32, kind="ExternalInput")
    w_d = nc.dram_tensor("kernel", [F, D], f32, kind="ExternalInput")
    o_d = nc.dram_tensor("out", [N, D], f32, kind="ExternalOutput")

    with tile.TileContext(nc) as tc, ExitStack() as ctx:
        const = ctx.enter_context(tc.tile_pool(name="const", bufs=1))
        xp = ctx.enter_context(tc.tile_pool(name="xp", bufs=1))
        wp = ctx.enter_context(tc.tile_pool(name="wp", bufs=1))
        xlp = ctx.enter_context(tc.tile_pool(name="xlp", bufs=4))
        afp = ctx.enter_context(tc.tile_pool(name="afp", bufs=3))
        abp = ctx.enter_context(tc.tile_pool(name="abp", bufs=6))
        atp = ctx.enter_context(tc.tile_pool(name="atp", bufs=2))
        ttp = ctx.enter_context(tc.tile_pool(name="ttp", bufs=2))
        outp = ctx.enter_context(tc.tile_pool(name="outp", bufs=3))
        scr = ctx.enter_context(tc.tile_pool(name="scr", bufs=2))
        ps_mm = ctx.enter_context(tc.tile_pool(name="ps_mm", bufs=4, space="PSUM"))
        ps_o = ctx.enter_context(tc.tile_pool(name="ps_o", bufs=2, space="PSUM"))
        ps_tp = ctx.enter_context(tc.tile_pool(name="ps_tp", bufs=2, space="PSUM"))

        ident = const.tile([P, P], f32)
        make_identity(nc, ident[:])
        ident_b = const.tile([P, P], bf16)
        nc.vector.tensor_copy(ident_b[:], ident[:])

        x_b = xp.tile([P, NT, F], bf16)
        w_b = wp.tile([P, FT, D], bf16)
        sumabs = const.tile([P, NT], f32)
        mask_sb = const.tile([P, NT], f32)

        def warm_fp32():
            pw = ps_o.tile([P, D], f32, tag="pso", name="pw")
            nc.tensor.matmul(
                pw[:, :P], lhsT=ident[:], rhs=ident[:], start=True, stop=True
            )

        def warm_bf16(lhs, rhs):
            # fires as the just-cast tile lands; paces PE activity (HAM)
            # through the DMA window.
            pw = ps_o.tile([P, D], f32, tag="pso", name="pwb")
            nc.tensor.matmul(
                pw[:, : rhs.shape[-1]], lhsT=lhs, rhs=rhs, start=True, stop=True
            )

        at_tiles = [None] * NJ

        def at_of(nj):
            if at_tiles[nj] is None:
                at_tiles[nj] = atp.tile(
                    [P, NT, NSUB, P], bf16, tag="at", name=f"at{nj}"
                )
            return at_tiles[nj]

        # ---------- preamble: loads (sync ring order = priority) ----------
        xls = [None] * NT

        def load_x(o):
            xl = xlp.tile([P, F], f32, tag="xl", name=f"xl{o}")
            nc.sync.dma_start(xl[:], x_d[o * P : (o + 1) * P, :])
            xls[o] = xl

        def load_strip(nj, ni):
            af = afp.tile([P, N], f32, tag="af", name="af")
            r0 = (nj * NSUB + ni) * P
            nc.sync.dma_start(af[:], a_d[r0 : r0 + P, :])
            return af

        for _ in range(10):
            warm_fp32()

        load_x(0)
        load_x(1)
        af0 = [load_strip(0, ni) for ni in range(NSUB)]
        for o in range(2, NT):
            load_x(o)
        af1 = [load_strip(1, ni) for ni in range(NSUB)]
        af2 = [load_strip(2, ni) for ni in range(NSUB)]
        af3 = [load_strip(3, ni) for ni in range(NSUB)]
        nc.gpsimd.dma_start(w_b[:], w_d[:].rearrange("(o p) d -> p o d", p=P))

        # ---------- preamble: chunk 0 cast + PE transpose ----------
        def cast_x(o):
            nc.vector.tensor_copy(x_b[:, o, :], xls[o][:])

        cast_x(0)
        cast_x(1)
        warm_bf16(x_b[:, 0, 0:P], x_b[:, 0, :])
        warm_bf16(x_b[:, 1, 0:P], x_b[:, 1, :])

        abs_ = {}  # (nj, ni) -> bf16 strip

        def cast_strip(nj, ni, af, warm=False):
            ab = abp.tile([P, N], bf16, tag="ab", name=f"ab{nj}_{ni}")
            nc.vector.tensor_copy(ab[:], af[:])
            if warm:
                warm_bf16(ab[:, 0:P], ab[:, 0:NCHUNK])
                warm_bf16(ab[:, P : 2 * P], ab[:, NCHUNK : 2 * NCHUNK])
            abs_[(nj, ni)] = ab

        cbn = 0  # copyback DVE/ACT alternation

        def t_quad(nj, slot):
            # PE-transpose 4 tiles (strip ni, m-tiles q*4..q*4+3) through one
            # PSUM bank, then DVE/ACT copy into at[p, mtile, ni, r].
            nonlocal cbn
            ni, q = divmod(slot, 4)
            ab = abs_[(nj, ni)]
            ps = ps_tp.tile([P, NCHUNK], bf16, tag="pst", name="pst")
            for k in range(4):
                mi = q * 4 + k
                nc.tensor.transpose(
                    ps[:, k * P : (k + 1) * P],
                    ab[:, mi * P : (mi + 1) * P],
                    ident_b[:],
                )
            src = ps[:].rearrange("p (a f) -> p a f", a=4)
            dst = at_of(nj)[:, q * 4 : (q + 1) * 4, ni, :]
            if cbn % 2 == 0:
                nc.vector.tensor_copy(dst, src)
            else:
                nc.scalar.copy(dst, src)
            cbn += 1

        for ni in range(NSUB):
            cast_strip(0, ni, af0[ni], warm=True)
            for q in range(4):
                t_quad(0, 4 * ni + q)

        for o in range(2, NT):
            cast_x(o)

        # mask: |x| row-sums via ACT (early in the ACT stream), mask =
        # (sum > 0) on DVE.
        for o in range(NT):
            abs_scr = scr.tile([P, F], bf16, tag="abs_scr")
            nc.scalar.activation(
                abs_scr[:], x_b[:, o, :], AF.Abs, accum_out=sumabs[:, o : o + 1]
            )
        nc.vector.tensor_scalar(
            mask_sb[:], sumabs[:], 0.0, None, mybir.AluOpType.is_gt
        )

        # ---------- main loop ----------
        tts = [None] * NJ

        def tt_copy(nj, pt, fi):
            if tts[nj] is None:
                tts[nj] = ttp.tile([P, FT, NCHUNK], bf16, tag="tt", name=f"tt{nj}")
            nc.scalar.copy(tts[nj][:, fi], pt[fi][:])

        def mm2_wave(nj, w):
            # half of mm2 for chunk nj: output tiles ns = 2w, 2w+1,
            # accumulated over fi in 2 ps_o banks, then fused
            # relu(mask * po) -> SBUF -> store (2 row-tiles per DMA).
            tt = tts[nj]
            pos = [
                ps_o.tile([P, D], f32, tag="pso", name=f"po_{nj}_{w}_{i}")
                for i in range(2)
            ]
            for fi in range(FT):
                for i in range(2):
                    ns = 2 * w + i
                    nc.tensor.matmul(
                        pos[i][:],
                        lhsT=tt[:, fi, ns * P : (ns + 1) * P],
                        rhs=w_b[:, fi],
                        start=(fi == 0),
                        stop=(fi == FT - 1),
                    )
            store_pair(nj, w, pos)

        def store_pair(nj, w, pos):
            ob = outp.tile([P, 2, D], f32, tag="ob")
            for i in range(2):
                ni = nj * NSUB + 2 * w + i
                nc.scalar.activation(
                    ob[:, i, :], pos[i][:], AF.Relu, scale=mask_sb[:, ni : ni + 1]
                )
            r0 = (nj * NSUB + 2 * w) * P
            dst = o_d[r0 : r0 + 2 * P, :].rearrange("(t p) d -> p t d", p=P)
            q = nc.scalar if nj < 2 else nc.sync
            q.dma_start(dst, ob[:])

        # --- chunk 0: mi-outer mm1 (x trickles in; at0 just finished) ---
        cast_strip(1, 0, af1[0])
        cast_strip(1, 1, af1[1])
        pt0 = [
            ps_mm.tile([P, NCHUNK], f32, tag="psm", name=f"pt_0_{fi}")
            for fi in range(FT)
        ]
        for mi in range(NT):
            for fi in range(FT):
                nc.tensor.matmul(
                    pt0[fi][:],
                    lhsT=x_b[:, mi, fi * P : (fi + 1) * P],
                    rhs=at_of(0)[:, mi, :, :],
                    start=(mi == 0),
                    stop=(mi == NT - 1),
                )
        for fi in range(FT):
            tt_copy(0, pt0, fi)

        # --- boundary c0->c1: c1 quads + mm2 c0 fill the at1 wait ---
        for slot in range(0, 8):       # strips 0,1 (cast early)
            t_quad(1, slot)
        mm2_wave(0, 0)
        cast_strip(1, 2, af1[2])
        for slot in range(8, 12):
            t_quad(1, slot)
        cast_strip(1, 3, af1[3])
        mm2_wave(0, 1)
        for slot in range(12, 16):
            t_quad(1, slot)

        # --- chunks 1..3: fi-outer; next chunk's cast after fi==ni group,
        #     its quads ride fi==ni+1; mm2 of nj-1 after fi==1 / fi==3 ---
        for nj in range(1, NJ):
            nxt = nj + 1 if nj + 1 < NJ else None
            af_n = (None, af2, af3, None)[nj]
            pt = [
                ps_mm.tile([P, NCHUNK], f32, tag="psm", name=f"pt_{nj}_{fi}")
                for fi in range(FT)
            ]
            for fi in range(FT):
                for mi in range(NT):
                    nc.tensor.matmul(
                        pt[fi][:],
                        lhsT=x_b[:, mi, fi * P : (fi + 1) * P],
                        rhs=at_of(nj)[:, mi, :, :],
                        start=(mi == 0),
                        stop=(mi == NT - 1),
                    )
                    # next chunk's quads ride this stream, one per 4 mms,
                    # lagging one fi group behind the strip's cast
                    if nxt is not None and fi >= 1 and mi % 4 == 3:
                        t_quad(nxt, (fi - 1) * 4 + mi // 4)
                tt_copy(nj, pt, fi)
                if nxt is not None and fi < NSUB:
                    cast_strip(nxt, fi, af_n[fi])
                if nj >= 2 and fi == 1:
                    mm2_wave(nj - 1, 0)
                elif nj >= 2 and fi == 3:
                    mm2_wave(nj - 1, 1)
            if nxt is not None:
                # last strip's quads after the fi loop
                for slot in range(12, 16):
                    t_quad(nxt, slot)

        # --- chunk 3 mm2: merged fi-major wave across 4 banks for a
        #     minimal tail (quads done -> ps_tp is free) ---
        pos3 = [
            (ps_o if i < 2 else ps_tp).tile(
                [P, D], f32, tag=("pso" if i < 2 else "pst"), name=f"po3_{i}"
            )
            for i in range(4)
        ]
        tt3 = tts[3]
        for fi in range(FT):
            for i in range(4):
                nc.tensor.matmul(
                    pos3[i][:],
                    lhsT=tt3[:, fi, i * P : (i + 1) * P],
                    rhs=w_b[:, fi],
                    start=(fi == 0),
                    stop=(fi == FT - 1),
                )
        store_pair(3, 0, pos3[0:2])
        store_pair(3, 1, pos3[2:4])

    nc.compile()
    return nc


def get_nc():
    if "nc" not in _CACHE:
        _CACHE["nc"] = _build_nc()
    return _CACHE["nc"]


def kernel(**inputs) -> np.ndarray:
    from concourse.bass_utils import run_bass_kernel_spmd

    x = np.ascontiguousarray(np.asarray(inputs["x"], dtype=np.float32))
    a = np.ascontiguousarray(np.asarray(inputs["a"], dtype=np.float32))
    w = np.ascontiguousarray(np.asarray(inputs["kernel"], dtype=np.float32))
    assert x.shape == (B, N, F) and a.shape == (B, N, N) and w.shape == (F, D)

    nc = get_nc()
    in_maps = [{"a": a[b], "x": x[b], "kernel": w} for b in range(B)]
    res = run_bass_kernel_spmd(nc, in_maps, core_ids=list(range(B)))
    return np.stack([res.results[b]["out"] for b in range(B)], axis=0)
